# revision 25
# baseline (speedup 1.0000x reference)
"""Trainium2 Bass kernel for CustomISTFT (N_FFT=4096, HOP=1024, T=4096 frames).

Per core (frames sharded 512/core across 8 cores), v2:
  Cooley-Tukey split of the 4096-point inverse DFT: k = 64*j1 + c,
  n = m1 + 64*m2.  Stage 1: one K=128 matmul per c (33 of them) against
  zall[(half, ch, j1), c, t] resident in SBUF; output A kept in SBUF
  with partition order (h, ri, q) where m1 = q + 32*h.  Corner turn 1
  is done ENTIRELY IN SBUF: per c, one DMA with a stride-31 partition
  view lands A[:, c, :] into the packed stage-2 rhs layout rt2
  [128 rows, 32 q, t] (rows = 62*h + 31*ri + (c-1), plus 4 edge rows
  for the rank-1 c in {0, 32} re-terms).  Stage 2 packs TWO m1 values
  (q and q+32) into one K=128 x M=128 matmul per q (32 matmuls), with
  conj-fold, window/3 and the c in {0,32} corrections all in w2p.
  Corner turn 2 likewise: per q one SBUF->SBUF DMA from xs[(par,m1h,nh)]
  partitions into x2[q::32] final OLA layout.  Overlap-add = 1 copy +
  3 shifted adds on DVE.  Imag channel is rank-2 (b0, b2048) and runs
  during the input load window.  No DRAM round trips for corner turns.
  z is pre-cast to bf16 and pre-gathered into zall on the host.
  Host: shard, gather, halo-add between neighbor cores, exact wsum
  correction on the two edge blocks.
"""

import numpy as np
import ml_dtypes

N_FFT = 4096
HOP = 1024
FREQ = 2049
T_FRAMES = 4096
N_CORES = 8
T_CORE = T_FRAMES // N_CORES  # 512
L_FULL = (T_FRAMES - 1) * HOP + N_FFT
OUT_LEN = L_FULL - N_FFT

_bf16 = ml_dtypes.bfloat16


# ---------------------------------------------------------------- weights
def build_weights(window):
    """w1 [128,33,128] bf16: row (64*half + 32*ch + j1) matches zall
    partition (half 0: k=c+64*j1, half 1: k=64-c+64*j1), col dim c,
    inner col = 64*h + 32*ri + q with m1 = q + 32*h (ri = re/im of A).
    w2p [128,32,128] bf16: packed stage-2 weights; row r = 62*h + 31*ri
    + (c-1) for c in 1..31 (plus rows 124+2h+e for the c=0/c=32 re
    rank-1 terms), pack-col q (m1 = q + 32*h), output col
    o = 64*par + 32*m1h + nh with m2 = 2*nh + par; window/3 folded.
    wim [8,1024] bf16 unchanged."""
    win = window.astype(np.float64)
    mu = np.exp(2j * np.pi / 4096)
    w64c = np.exp(2j * np.pi / 64)
    m1v = np.arange(64)

    w1 = np.zeros((128, 33, 128), dtype=np.float64)
    for g in range(33):
        coef = {}
        for j1 in range(64):
            k = 64 * j1 + g
            e = w64c ** (m1v * j1)
            if k <= 2048:
                coef[(0, k)] = coef.get((0, k), 0) + e
                coef[(1, k)] = coef.get((1, k), 0) + 1j * e
            else:
                kr = 4096 - k
                coef[(0, kr)] = coef.get((0, kr), 0) + e
                coef[(1, kr)] = coef.get((1, kr), 0) - 1j * e
        for (ch, k), v0 in coef.items():
            v = v0 * mu ** (m1v * g)
            rows = []
            jA, rA = divmod(k - g, 64)
            if rA == 0 and 0 <= jA < 32:
                rows.append(32 * ch + jA)
            jB, rB = divmod(k - (64 - g), 64)
            if rB == 0 and 0 <= jB < 32:
                rows.append(64 + 32 * ch + jB)
            assert rows, (g, ch, k)
            for row in rows:
                w1[row, g, :64] += v.real / len(rows)
                w1[row, g, 64:] += v.imag / len(rows)
    # permute columns: old col = 64*ri + m1  ->  new col = 64*h + 32*ri + q
    perm = np.zeros(128, dtype=np.int64)
    for ri in range(2):
        for m1 in range(64):
            h, q = divmod(m1, 32)
            perm[64 * h + 32 * ri + q] = 64 * ri + m1
    w1 = w1[:, :, perm]

    # stage 2 packed: x[m1 + 64*m2] = (1/4096) * [ ReA[m1,0]
    #   + (-1)^m2 ReA[m1,32] + sum_{c=1..31} 2*(cos(th) ReA[m1,c]
    #   - sin(th) ImA[m1,c]) ], th = 2*pi*m2*c/64, times win[n]*4096/3.
    # K-row r = 4*(c-1) + 2*h + ri for c in 1..31 (matches the corner-turn-1
    # fan of A partitions (h, ri, q) -> rt2[4*(c-1) + j]), edge rows
    # 124+h (c=0 re) and 126+h (c=32 re).
    w2p = np.zeros((128, 32, 128), dtype=np.float64)
    nh_v = np.arange(32)
    for h in range(2):
        for q in range(32):
            m1 = q + 32 * h
            for par in range(2):
                o = 64 * par + 32 * h + nh_v  # [32]
                m2 = 2 * nh_v + par
                n = m1 + 64 * m2
                wn = win[n] / 3.0
                for c in range(1, 32):
                    th = 2.0 * np.pi * m2 * c / 64.0
                    w2p[4 * (c - 1) + 2 * h + 0, q, o] = 2.0 * np.cos(th) * wn
                    w2p[4 * (c - 1) + 2 * h + 1, q, o] = -2.0 * np.sin(th) * wn
                w2p[124 + h, q, o] = wn
                w2p[126 + h, q, o] = np.where(m2 % 2 == 0, wn, -wn)

    # wim[(2r+par), i] = win[i + 1024 r]/3 * (par == i%2)
    wim = np.zeros((8, 1024), dtype=np.float64)
    iv = np.arange(1024)
    for r in range(4):
        for par in range(2):
            wim[2 * r + par] = (win[iv + 1024 * r] / 3.0) * (iv % 2 == par)
    return w1.astype(_bf16), w2p.astype(_bf16), wim.astype(_bf16)


# ---------------------------------------------------------------- device program
def emit_kernel(tc, outre_ap, outim_ap, zall_ap, zedge_ap, w1_ap, w2p_ap, wim_ap, T,
                debug_aps=None):
    """Per-core program.  T frames (multiple of 128).
    outre [128, 8, SPAD] bf16:  outre[p, ih, s] =
        sum_r win*x[p + 128*ih + 1024*r, s - r]  (real channel, s in [0,T+3))
    outim [128, SC, 1024] bf16: outim[sp, sc, i] = imag channel at block
        s = 128*sc + sp, position i."""
    import concourse.mybir as mybir
    from contextlib import ExitStack

    nc = tc.nc
    dt = mybir.dt
    f32, bf16 = dt.float32, dt.bfloat16
    SB = T + 3
    SC = (SB + 127) // 128
    SPAD = outre_ap.shape[2]
    assert SPAD >= SB and outim_ap.shape[1] == SC

    with ExitStack() as ctx:
        const = ctx.enter_context(tc.tile_pool(name="const", bufs=1))

        # persistent tiles
        sig = const.tile([128, 8, SPAD], bf16)
        w2p_sb = const.tile([128, 32, 128], bf16)
        wim_sb = const.tile([8, 1024], bf16)
        cve = const.tile([1, T], bf16)
        cvo = const.tile([1, T], bf16)
        b0t = const.tile([1, T], bf16)
        b2t = const.tile([1, T], bf16)
        cs = const.tile([8, SC * 128], bf16)
        imall = const.tile([128, SC, 1024], bf16)
        # packed stage-2 rhs; free dim padded to 33 so the partition dim of
        # multi-partition DMA dst views cannot be merged away (bass AP-opt
        # merges partition into free when strides line up, which lowers to
        # wrong/illegal SBUF descriptors)
        rt2 = const.tile([128, 33, T], bf16)

        nc.any.memset(cs[:], 0.0)

        # b0 = z[1,0,:], b2048 = z[1,2048,:];  CS[2r+par, s] = cv_par[s-r]
        nc.scalar.dma_start(b0t[:], zedge_ap[0:1, :])
        nc.scalar.dma_start(b2t[:], zedge_ap[1:2, :])
        nc.vector.tensor_add(cve[:], b0t[:], b2t[:])
        nc.vector.tensor_sub(cvo[:], b0t[:], b2t[:])
        for r in range(4):
            nc.scalar.dma_start(cs[2 * r : 2 * r + 1, r : r + T], cve[:])
            nc.scalar.dma_start(cs[2 * r + 1 : 2 * r + 2, r : r + T], cvo[:])

        # ---- phase 1: chunked zall/w1 loads overlap 33 stage-1 matmuls;
        #      corner turn 1 lands straight into rt2 via SBUF->SBUF DMAs.
        #      First chunk is small so the first matmul can start early;
        #      w1 leads on gpsimd (weights for g=0 gate the tensor queue).
        GCH = [(0, 6), (6, 15), (15, 24), (24, 33)]
        dmaq = [nc.sync, nc.scalar, nc.gpsimd]

        # SBUF->SBUF DMAs on the two HWDGE queues land on a fixed 4-engine
        # group (64-67); only the gpsimd software DGE fans them across all
        # 16 engines.  Corner turns therefore go mostly through gpsimd.
        def ct1(c, eng):
            if c == 0 or c == 32:
                e = 0 if c == 0 else 2
                for h in range(2):
                    eng.dma_start(
                        rt2[124 + e + h : 125 + e + h, 0:32, :],
                        A[64 * h : 64 * h + 32, c, :],
                    )
            else:
                # fan A[:, c, :] (128 partitions, p = 32j + q) into 4
                # contiguous rt2 partitions 4(c-1)+j, free (q, t)
                eng.dma_start(
                    rt2[4 * (c - 1) : 4 * (c - 1) + 4, 0:32, :], A[:, c, :]
                )

        ct1q = [nc.gpsimd, nc.gpsimd, nc.sync]
        with (
            tc.tile_pool(name="ph1", bufs=1) as ph1,
            tc.tile_pool(name="s1ps", bufs=3, space="PSUM") as s1ps,
        ):
            zall_sb = ph1.tile([128, 33, T], bf16)
            w1_sb = ph1.tile([128, 33, 128], bf16)
            A = ph1.tile([128, 33, T], bf16)  # [(h ri q), c, t]
            for i, (g0, g1) in enumerate(GCH):
                nc.gpsimd.dma_start(w1_sb[:, g0:g1, :], w1_ap[:, g0:g1, :])
                (nc.sync if i % 2 == 0 else nc.scalar).dma_start(
                    zall_sb[:, g0:g1, :], zall_ap[:, g0:g1, :]
                )
            # stage-2/imag weights after w1 so they don't gate stage-1;
            # on the HW queues (DRAM loads stripe wide there) to keep the
            # gpsimd queue free for corner-turn traffic
            nc.scalar.dma_start(wim_sb[:], wim_ap[:])
            nc.scalar.dma_start(w2p_sb[:], w2p_ap[:])

            for gp in range(17):
                g0 = 2 * gp
                gn = min(2, 33 - g0)
                ps = s1ps.tile([128, 1024], f32, tag="s1ps")
                for gi in range(gn):
                    g = g0 + gi
                    nc.tensor.matmul(
                        ps[:, 512 * gi : 512 * (gi + 1)],
                        w1_sb[:, g, :],
                        zall_sb[:, g, :],
                        start=True,
                        stop=True,
                    )
                cp = nc.vector.tensor_copy if gp % 2 else nc.scalar.copy
                cp(A[:, g0 : g0 + gn, :], ps[:, : 512 * gn])
                for gi in range(gn):
                    g = g0 + gi
                    ct1(g, ct1q[g % len(ct1q)])
            # imag channel (rank-2): fills the tensor queue while corner
            # turn 1 drains (it depends only on cs/wim)
            with tc.tile_pool(name="imps", bufs=2, space="PSUM") as impool:
                for sc in range(SC):
                    for half in range(2):
                        ips = impool.tile([128, 512], f32, tag="imps")
                        nc.tensor.matmul(
                            ips[:],
                            cs[:, sc * 128 : (sc + 1) * 128],
                            wim_sb[:, 512 * half : 512 * (half + 1)],
                            start=True,
                            stop=True,
                        )
                        nc.any.tensor_copy(
                            imall[:, sc, 512 * half : 512 * (half + 1)], ips[:]
                        )
            nc.scalar.dma_start(outim_ap[:], imall[:], max_dma_last_dim=512)
            if debug_aps is not None:
                nc.gpsimd.dma_start(debug_aps["A"], A[:])

        # ---- phase 2: 32 packed matmuls, corner turn 2 SBUF->SBUF, OLA
        phx = ctx.enter_context(tc.tile_pool(name="phx", bufs=1))
        s2ps = ctx.enter_context(tc.tile_pool(name="s2ps", bufs=3, space="PSUM"))
        xs = phx.tile([128, 32, T], bf16)  # [(par m1h nh), q, t]
        x2 = phx.tile([128, 33, T], bf16)  # [(4q + 2par + m1h), nh, t]; padded
        tmp1 = phx.tile([128, 8, T + 1], bf16)
        tmp2 = phx.tile([128, 8, T + 1], bf16)

        for qp in range(16):
            q0 = 2 * qp
            ps2 = s2ps.tile([128, 1024], f32, tag="s2ps")
            for qi in range(2):
                q = q0 + qi
                nc.tensor.matmul(
                    ps2[:, 512 * qi : 512 * (qi + 1)],
                    w2p_sb[:, q, :],
                    rt2[:, q, :],
                    start=True,
                    stop=True,
                )
            cp = nc.vector.tensor_copy if qp % 2 else nc.scalar.copy
            cp(xs[:, q0 : q0 + 2, :], ps2[:])
            for qi in range(2):
                q = q0 + qi
                # fan xs[:, q, :] (partition o = 32u + nh) into 4 contiguous
                # x2 partitions 4q + u, free (nh, t)
                (nc.gpsimd if qi == 0 else nc.sync).dma_start(
                    x2[4 * q : 4 * q + 4, 0:32, :], xs[:, q, :]
                )

        if debug_aps is not None:
            nc.gpsimd.dma_start(debug_aps["rt2"], rt2[:])
            nc.gpsimd.dma_start(debug_aps["xs"], xs[:])
            nc.gpsimd.dma_start(debug_aps["x2"], x2[:])

        # ---- OLA (real), 2-level tree: tmp1 = r0 + r1<<1, tmp2 = r2 + r3<<1
        #      (on vector and gpsimd in parallel), sig = tmp1 + tmp2<<2.
        #      tmpX[s] covers s in [0, T]; sig[s] valid for s in [0, T+3).
        nc.vector.tensor_copy(tmp1[:, :, 0:1], x2[:, 0:8, 0:1])
        nc.vector.tensor_add(
            tmp1[:, :, 1:T], x2[:, 0:8, 1:T], x2[:, 8:16, 0 : T - 1]
        )
        nc.vector.tensor_copy(tmp1[:, :, T : T + 1], x2[:, 8:16, T - 1 : T])
        nc.vector.tensor_copy(tmp2[:, :, 0:1], x2[:, 16:24, 0:1])
        nc.vector.tensor_add(
            tmp2[:, :, 1:T], x2[:, 16:24, 1:T], x2[:, 24:32, 0 : T - 1]
        )
        nc.vector.tensor_copy(tmp2[:, :, T : T + 1], x2[:, 24:32, T - 1 : T])
        nc.vector.tensor_copy(sig[:, :, 0:2], tmp1[:, :, 0:2])
        nc.vector.tensor_add(
            sig[:, :, 2 : T + 1], tmp1[:, :, 2 : T + 1], tmp2[:, :, 0 : T - 1]
        )
        nc.vector.tensor_copy(
            sig[:, :, T + 1 : T + 3], tmp2[:, :, T - 1 : T + 1]
        )
        nc.sync.dma_start(outre_ap[:], sig[:], max_dma_last_dim=260)


# ---------------------------------------------------------------- build + run
_CACHE = {}
SPAD = 520  # padded s extent of outre (>= T_CORE + 3)


def _build(T):
    import concourse.bacc as bacc
    import concourse.tile as tile
    import concourse.mybir as mybir

    dt = mybir.dt
    SC = (T + 3 + 127) // 128
    nc = bacc.Bacc("TRN2", target_bir_lowering=False, debug=False, num_devices=N_CORES)
    zall_t = nc.dram_tensor("zall", [128, 33, T], dt.bfloat16, kind="ExternalInput")
    zedge_t = nc.dram_tensor("zedge", [2, T], dt.bfloat16, kind="ExternalInput")
    w1_t = nc.dram_tensor("w1", [128, 33, 128], dt.bfloat16, kind="ExternalInput")
    w2p_t = nc.dram_tensor("w2p", [128, 32, 128], dt.bfloat16, kind="ExternalInput")
    wim_t = nc.dram_tensor("wim", [8, 1024], dt.bfloat16, kind="ExternalInput")
    spad = max(SPAD, T + 3)
    outre_t = nc.dram_tensor(
        "outre", [128, 8, spad], dt.bfloat16, kind="ExternalOutput"
    )
    outim_t = nc.dram_tensor(
        "outim", [128, SC, 1024], dt.bfloat16, kind="ExternalOutput"
    )
    with tile.TileContext(nc) as tc:
        emit_kernel(
            tc, outre_t.ap(), outim_t.ap(), zall_t.ap(),
            zedge_t.ap(), w1_t.ap(), w2p_t.ap(), wim_t.ap(), T,
        )
    nc.compile()
    return nc


# outre partition order is p' = 4*q + 2*par + m1h (p = m1 + 64*par,
# m1 = q + 32*m1h); PP[p] = p' un-permutes on the host
PP = np.empty(128, dtype=np.int64)
for _p in range(128):
    _par, _m1 = _p >> 6, _p & 63
    PP[_p] = 4 * (_m1 & 31) + 2 * _par + (_m1 >> 5)


def core_out_to_sig(outre, outim, T):
    """[128,8,spad] bf16 + [128,SC,1024] bf16 -> [2, (T+3)*1024] f32."""
    SB = T + 3
    re = (
        np.asarray(outre, dtype=np.float32)[PP]
        .transpose(2, 1, 0)
        .reshape(-1, 1024)[:SB]
    )
    im = (
        np.asarray(outim, dtype=np.float32)
        .transpose(1, 0, 2)
        .reshape(-1, 1024)[:SB]
    )
    return np.stack([re.reshape(-1), im.reshape(-1)])


_KIDXA = np.arange(33)[None, :] + 64 * np.arange(32)[:, None]  # [32, 33]
_KIDXB = (64 - np.arange(33))[None, :] + 64 * np.arange(32)[:, None]  # [32, 33]


def make_in_maps(z, window):
    """Shard full f32 inputs into per-core bf16 in_maps."""
    zb = np.asarray(z, dtype=np.float32).astype(_bf16)
    wkey = window.tobytes()
    if _CACHE.get("wkey") != wkey:
        _CACHE["weights"] = build_weights(np.asarray(window, dtype=np.float32))
        _CACHE["wkey"] = wkey
    w1, w2p, wim = _CACHE["weights"]
    in_maps = []
    for m in range(N_CORES):
        zc = zb[:, :, m * T_CORE : (m + 1) * T_CORE]
        zall = np.empty((128, 33, T_CORE), dtype=_bf16)
        zall[0:64] = zc[:, _KIDXA, :].reshape(64, 33, T_CORE)
        zall[64:128] = zc[:, _KIDXB, :].reshape(64, 33, T_CORE)
        zedge = np.ascontiguousarray(zc[1, [0, 2048], :])  # [2, T]
        in_maps.append(
            {"zall": zall, "zedge": zedge, "w1": w1, "w2p": w2p, "wim": wim}
        )
    return in_maps


def kernel(z, window):
    from concourse.bass_utils import run_bass_kernel_spmd

    z = np.asarray(z, dtype=np.float32)
    window = np.asarray(window, dtype=np.float32)
    assert z.shape == (2, FREQ, T_FRAMES)

    if "nc" not in _CACHE:
        _CACHE["nc"] = _build(T_CORE)
    nc = _CACHE["nc"]

    in_maps = make_in_maps(z, window)
    res = run_bass_kernel_spmd(nc, in_maps, core_ids=list(range(N_CORES)))

    full = np.zeros((2, L_FULL), dtype=np.float32)
    span = (T_CORE + 3) * 1024
    for m in range(N_CORES):
        o = core_out_to_sig(res.results[m]["outre"], res.results[m]["outim"], T_CORE)
        full[:, m * T_CORE * HOP : m * T_CORE * HOP + span] += o
    out = full[:, N_FFT // 2 : L_FULL - N_FFT // 2]

    win = window.astype(np.float64)
    ws_start = win[0:1024] + win[1024:2048] + win[2048:3072]
    ws_end = win[1024:2048] + win[2048:3072] + win[3072:4096]
    out[:, :1024] *= ((3.0 / 4096.0) / ws_start).astype(np.float32)[None, :]
    out[:, -1024:] *= ((3.0 / 4096.0) / ws_end).astype(np.float32)[None, :]
    return out


# revision 32
# speedup vs baseline: 1.3102x; 1.3102x over previous
"""Trainium2 Bass kernel for CustomISTFT (N_FFT=4096, HOP=1024, T=4096 frames).

Per core (frames sharded 512/core across 8 cores), v2:
  Cooley-Tukey split of the 4096-point inverse DFT: k = 64*j1 + c,
  n = m1 + 64*m2.  Stage 1: one K=128 matmul per c (33 of them) against
  zall[(half, ch, j1), c, t] resident in SBUF; output A kept in SBUF
  with partition order (h, ri, q) where m1 = q + 32*h.  Corner turn 1
  is done ENTIRELY IN SBUF: per c, one DMA with a stride-31 partition
  view lands A[:, c, :] into the packed stage-2 rhs layout rt2
  [128 rows, 32 q, t] (rows = 62*h + 31*ri + (c-1), plus 4 edge rows
  for the rank-1 c in {0, 32} re-terms).  Stage 2 packs TWO m1 values
  (q and q+32) into one K=128 x M=128 matmul per q (32 matmuls), with
  conj-fold, window/3 and the c in {0,32} corrections all in w2p.
  Corner turn 2 likewise: per q one SBUF->SBUF DMA from xs[(par,m1h,nh)]
  partitions into x2[q::32] final OLA layout.  Overlap-add = 1 copy +
  3 shifted adds on DVE.  Imag channel is rank-2 (b0, b2048) and runs
  during the input load window.  No DRAM round trips for corner turns.
  z is pre-cast to bf16 and pre-gathered into zall on the host.
  Host: shard, gather, halo-add between neighbor cores, exact wsum
  correction on the two edge blocks.
"""

import numpy as np
import ml_dtypes

N_FFT = 4096
HOP = 1024
FREQ = 2049
T_FRAMES = 4096
N_CORES = 8
T_CORE = T_FRAMES // N_CORES  # 512
L_FULL = (T_FRAMES - 1) * HOP + N_FFT
OUT_LEN = L_FULL - N_FFT

_bf16 = ml_dtypes.bfloat16


# ---------------------------------------------------------------- weights
def build_weights(window):
    """w1 [128,33,128] bf16: row (64*half + 32*ch + j1) matches zall
    partition (half 0: k=c+64*j1, half 1: k=64-c+64*j1), col dim c,
    inner col = 64*h + 32*ri + q with m1 = q + 32*h (ri = re/im of A).
    w2p [128,32,128] bf16: packed stage-2 weights; row r = 62*h + 31*ri
    + (c-1) for c in 1..31 (plus rows 124+2h+e for the c=0/c=32 re
    rank-1 terms), pack-col q (m1 = q + 32*h), output col
    o = 64*par + 32*m1h + nh with m2 = 2*nh + par; window/3 folded.
    wim [8,1024] bf16 unchanged."""
    win = window.astype(np.float64)
    mu = np.exp(2j * np.pi / 4096)
    w64c = np.exp(2j * np.pi / 64)
    m1v = np.arange(64)

    w1 = np.zeros((128, 33, 128), dtype=np.float64)
    for g in range(33):
        coef = {}
        for j1 in range(64):
            k = 64 * j1 + g
            e = w64c ** (m1v * j1)
            if k <= 2048:
                coef[(0, k)] = coef.get((0, k), 0) + e
                coef[(1, k)] = coef.get((1, k), 0) + 1j * e
            else:
                kr = 4096 - k
                coef[(0, kr)] = coef.get((0, kr), 0) + e
                coef[(1, kr)] = coef.get((1, kr), 0) - 1j * e
        for (ch, k), v0 in coef.items():
            v = v0 * mu ** (m1v * g)
            rows = []
            jA, rA = divmod(k - g, 64)
            if rA == 0 and 0 <= jA < 32:
                rows.append(32 * ch + jA)
            jB, rB = divmod(k - (64 - g), 64)
            if rB == 0 and 0 <= jB < 32:
                rows.append(64 + 32 * ch + jB)
            assert rows, (g, ch, k)
            for row in rows:
                w1[row, g, :64] += v.real / len(rows)
                w1[row, g, 64:] += v.imag / len(rows)
    # permute columns: old col = 64*ri + m1  ->  new col = 64*h + 32*ri + q
    perm = np.zeros(128, dtype=np.int64)
    for ri in range(2):
        for m1 in range(64):
            h, q = divmod(m1, 32)
            perm[64 * h + 32 * ri + q] = 64 * ri + m1
    w1 = w1[:, :, perm]

    # stage 2 packed: x[m1 + 64*m2] = (1/4096) * [ ReA[m1,0]
    #   + (-1)^m2 ReA[m1,32] + sum_{c=1..31} 2*(cos(th) ReA[m1,c]
    #   - sin(th) ImA[m1,c]) ], th = 2*pi*m2*c/64, times win[n]*4096/3.
    # K-row r = 4*(c-1) + 2*h + ri for c in 1..31 (matches the corner-turn-1
    # fan of A partitions (h, ri, q) -> rt2[4*(c-1) + j]), edge rows
    # 124+h (c=0 re) and 126+h (c=32 re).
    w2p = np.zeros((128, 32, 128), dtype=np.float64)
    nh_v = np.arange(32)
    for h in range(2):
        for q in range(32):
            m1 = q + 32 * h
            for par in range(2):
                o = 64 * par + 32 * h + nh_v  # [32]
                m2 = 2 * nh_v + par
                n = m1 + 64 * m2
                wn = win[n] / 3.0
                for c in range(1, 32):
                    th = 2.0 * np.pi * m2 * c / 64.0
                    w2p[4 * (c - 1) + 2 * h + 0, q, o] = 2.0 * np.cos(th) * wn
                    w2p[4 * (c - 1) + 2 * h + 1, q, o] = -2.0 * np.sin(th) * wn
                w2p[124 + h, q, o] = wn
                w2p[126 + h, q, o] = np.where(m2 % 2 == 0, wn, -wn)

    # wim[(2r+par), i] = win[i + 1024 r]/3 * (par == i%2)
    wim = np.zeros((8, 1024), dtype=np.float64)
    iv = np.arange(1024)
    for r in range(4):
        for par in range(2):
            wim[2 * r + par] = (win[iv + 1024 * r] / 3.0) * (iv % 2 == par)
    return w1.astype(_bf16), w2p.astype(_bf16), wim.astype(_bf16)


# ---------------------------------------------------------------- device program
def emit_kernel(tc, outre_ap, outim_ap, zall_ap, zedge_ap, w1_ap, w2p_ap, wim_ap, T,
                debug_aps=None):
    """Per-core program.  T frames (multiple of 128).
    outre [128, 8, SPAD] bf16:  outre[p, ih, s] =
        sum_r win*x[p + 128*ih + 1024*r, s - r]  (real channel, s in [0,T+3))
    outim [128, SC, 1024] bf16: outim[sp, sc, i] = imag channel at block
        s = 128*sc + sp, position i."""
    import concourse.mybir as mybir
    from contextlib import ExitStack

    nc = tc.nc
    dt = mybir.dt
    f32, bf16 = dt.float32, dt.bfloat16
    SB = T + 3
    SC = (SB + 127) // 128
    SPAD = outre_ap.shape[2]
    assert SPAD >= SB and outim_ap.shape[1] == SC

    with ExitStack() as ctx:
        const = ctx.enter_context(tc.tile_pool(name="const", bufs=1))
        dram = ctx.enter_context(tc.tile_pool(name="dram", bufs=1, space="DRAM"))

        # persistent tiles
        sig = const.tile([128, 8, SPAD], bf16)
        w2p_sb = const.tile([128, 32, 128], bf16)
        wim_sb = const.tile([8, 1024], bf16)
        cve = const.tile([1, T], bf16)
        cvo = const.tile([1, T], bf16)
        b0t = const.tile([1, T], bf16)
        b2t = const.tile([1, T], bf16)
        cs = const.tile([8, SC * 128], bf16)
        imall = const.tile([128, SC, 1024], bf16)
        # packed stage-2 rhs; free dim padded to 33 so the partition dim of
        # multi-partition DMA dst views cannot be merged away (bass AP-opt
        # merges partition into free when strides line up, which lowers to
        # wrong/illegal SBUF descriptors)
        rt2 = const.tile([128, 33, T], bf16)

        nc.any.memset(cs[:], 0.0)

        # b0 = z[1,0,:], b2048 = z[1,2048,:];  CS[2r+par, s] = cv_par[s-r]
        # (first sync-queue work so the imag matmuls unblock early)
        nc.sync.dma_start(b0t[:], zedge_ap[0:1, :])
        nc.sync.dma_start(b2t[:], zedge_ap[1:2, :])
        nc.vector.tensor_add(cve[:], b0t[:], b2t[:])
        nc.vector.tensor_sub(cvo[:], b0t[:], b2t[:])
        for r in range(4):
            nc.sync.dma_start(cs[2 * r : 2 * r + 1, r : r + T], cve[:])
            nc.sync.dma_start(cs[2 * r + 1 : 2 * r + 2, r : r + T], cvo[:])

        # ---- phase 1: chunked zall/w1 loads overlap 33 stage-1 matmuls;
        #      corner turn 1 lands straight into rt2 via SBUF->SBUF DMAs.
        #      First chunk is small so the first matmul can start early;
        #      w1 leads on gpsimd (weights for g=0 gate the tensor queue).
        GCH = [(0, 6), (6, 15), (15, 24), (24, 33)]
        dmaq = [nc.sync, nc.scalar, nc.gpsimd]

        # Corner-turn transport: SBUF->SBUF fan DMAs on the HWDGE queues
        # land on a fixed shared 4-engine group (~48 GB/s total) and the
        # gpsimd software DGE sustains ~40 GB/s, so the bulk of each corner
        # turn goes through DRAM instead: the 1KB-run fan happens on the
        # WRITE side (HBM writes stripe across all 16 engines by channel)
        # and comes back as wide contiguous reads.
        rt2d = dram.tile([128, 32, T], bf16)
        x2d = dram.tile([128, 32, T], bf16)
        CT1_SB = 28  # c >= CT1_SB (plus c=0/32 edges) stay SBUF->SBUF
        CT2_SB = 28  # q >= CT2_SB stay SBUF->SBUF

        def ct1(c, eng):
            if c == 0 or c == 32:
                e = 0 if c == 0 else 2
                for h in range(2):
                    nc.gpsimd.dma_start(
                        rt2[124 + e + h : 125 + e + h, 0:32, :],
                        A[64 * h : 64 * h + 32, c, :],
                    )
            elif c >= CT1_SB:
                # fan A[:, c, :] (128 partitions, p = 32j + q) into 4
                # contiguous rt2 partitions 4(c-1)+j, free (q, t)
                nc.gpsimd.dma_start(
                    rt2[4 * (c - 1) : 4 * (c - 1) + 4, 0:32, :], A[:, c, :]
                )
            else:
                eng.dma_start(
                    rt2d[4 * (c - 1) : 4 * (c - 1) + 4, :, :], A[:, c, :]
                )
            # chunked readback of DRAM-routed rows once their c-range landed
            if 1 <= c < CT1_SB and (c % 7 == 0 or c == CT1_SB - 1):
                r1 = 4 * c
                r0 = max(0, r1 - 28) if c % 7 == 0 else 4 * ((c // 7) * 7)
                eng.dma_start(rt2[r0:r1, 0:32, :], rt2d[r0:r1, :, :])

        ct1q = [nc.sync, nc.scalar]
        with (
            tc.tile_pool(name="ph1", bufs=1) as ph1,
            tc.tile_pool(name="s1ps", bufs=3, space="PSUM") as s1ps,
        ):
            # separate tiles per load chunk: cross-queue subtile deps are
            # conservative, one big tile would stall stage-1 on ALL chunks
            zall_sb = [
                ph1.tile([128, g1 - g0, T], bf16, name=f"zc{i}")
                for i, (g0, g1) in enumerate(GCH)
            ]
            w1_sb = [
                ph1.tile([128, g1 - g0, 128], bf16, name=f"wc{i}")
                for i, (g0, g1) in enumerate(GCH)
            ]
            A = ph1.tile([128, 33, T], bf16)  # [(h ri q), c, t]
            for i, (g0, g1) in enumerate(GCH):
                nc.gpsimd.dma_start(w1_sb[i][:], w1_ap[:, g0:g1, :])
                (nc.sync if i % 2 == 0 else nc.scalar).dma_start(
                    zall_sb[i][:], zall_ap[:, g0:g1, :]
                )
            # stage-2/imag weights after w1 so they don't gate stage-1;
            # on the HW queues (DRAM loads stripe wide there) to keep the
            # gpsimd queue free for corner-turn traffic
            nc.scalar.dma_start(wim_sb[:], wim_ap[:])
            nc.scalar.dma_start(w2p_sb[:], w2p_ap[:])

            def gslice(g):
                for i, (g0, g1) in enumerate(GCH):
                    if g0 <= g < g1:
                        return i, g - g0
                raise AssertionError(g)

            for gp in range(17):
                g0 = 2 * gp
                gn = min(2, 33 - g0)
                ps = s1ps.tile([128, 1024], f32, tag="s1ps")
                for gi in range(gn):
                    g = g0 + gi
                    ci, gl = gslice(g)
                    nc.tensor.matmul(
                        ps[:, 512 * gi : 512 * (gi + 1)],
                        w1_sb[ci][:, gl, :],
                        zall_sb[ci][:, gl, :],
                        start=True,
                        stop=True,
                    )
                cp = nc.vector.tensor_copy if gp % 2 else nc.scalar.copy
                cp(A[:, g0 : g0 + gn, :], ps[:, : 512 * gn])
                for gi in range(gn):
                    g = g0 + gi
                    ct1(g, ct1q[g % len(ct1q)])
            # imag channel (rank-2): fills the tensor queue while corner
            # turn 1 drains (it depends only on cs/wim)
            with tc.tile_pool(name="imps", bufs=2, space="PSUM") as impool:
                for sc in range(SC):
                    for half in range(2):
                        ips = impool.tile([128, 512], f32, tag="imps")
                        nc.tensor.matmul(
                            ips[:],
                            cs[:, sc * 128 : (sc + 1) * 128],
                            wim_sb[:, 512 * half : 512 * (half + 1)],
                            start=True,
                            stop=True,
                        )
                        nc.any.tensor_copy(
                            imall[:, sc, 512 * half : 512 * (half + 1)], ips[:]
                        )
            nc.scalar.dma_start(outim_ap[:], imall[:], max_dma_last_dim=512)
            if debug_aps is not None:
                nc.gpsimd.dma_start(debug_aps["A"], A[:])

        # ---- phase 2: 32 packed matmuls, corner turn 2 SBUF->SBUF, OLA
        phx = ctx.enter_context(tc.tile_pool(name="phx", bufs=1))
        s2ps = ctx.enter_context(tc.tile_pool(name="s2ps", bufs=3, space="PSUM"))
        xs = phx.tile([128, 32, T], bf16)  # [(par m1h nh), q, t]
        x2 = phx.tile([128, 33, T], bf16)  # [(4q + 2par + m1h), nh, t]; padded
        tmp1 = phx.tile([128, 8, T + 1], bf16)
        tmp2 = phx.tile([128, 8, T + 1], bf16)

        for qp in range(16):
            q0 = 2 * qp
            ps2 = s2ps.tile([128, 1024], f32, tag="s2ps")
            for qi in range(2):
                q = q0 + qi
                nc.tensor.matmul(
                    ps2[:, 512 * qi : 512 * (qi + 1)],
                    w2p_sb[:, q, :],
                    rt2[:, q, :],
                    start=True,
                    stop=True,
                )
            cp = nc.vector.tensor_copy if qp % 2 else nc.scalar.copy
            cp(xs[:, q0 : q0 + 2, :], ps2[:])
            for qi in range(2):
                q = q0 + qi
                # fan xs[:, q, :] (partition o = 32u + nh) into 4 contiguous
                # x2 partitions 4q + u, free (nh, t)
                if q >= CT2_SB:
                    nc.gpsimd.dma_start(
                        x2[4 * q : 4 * q + 4, 0:32, :], xs[:, q, :]
                    )
                else:
                    (nc.sync if qi == 0 else nc.scalar).dma_start(
                        x2d[4 * q : 4 * q + 4, :, :], xs[:, q, :]
                    )
                    if q % 7 == 6:
                        r0, r1 = 4 * (q - 6), 4 * (q + 1)
                        (nc.scalar if qi == 0 else nc.sync).dma_start(
                            x2[r0:r1, 0:32, :], x2d[r0:r1, :, :]
                        )

        if debug_aps is not None:
            nc.gpsimd.dma_start(debug_aps["rt2"], rt2[:])
            nc.gpsimd.dma_start(debug_aps["xs"], xs[:])
            nc.gpsimd.dma_start(debug_aps["x2"], x2[:])

        # ---- OLA (real), 2-level tree: tmp1 = r0 + r1<<1, tmp2 = r2 + r3<<1
        #      (on vector and gpsimd in parallel), sig = tmp1 + tmp2<<2.
        #      tmpX[s] covers s in [0, T]; sig[s] valid for s in [0, T+3).
        nc.vector.tensor_copy(tmp1[:, :, 0:1], x2[:, 0:8, 0:1])
        nc.vector.tensor_add(
            tmp1[:, :, 1:T], x2[:, 0:8, 1:T], x2[:, 8:16, 0 : T - 1]
        )
        nc.vector.tensor_copy(tmp1[:, :, T : T + 1], x2[:, 8:16, T - 1 : T])
        nc.vector.tensor_copy(tmp2[:, :, 0:1], x2[:, 16:24, 0:1])
        nc.vector.tensor_add(
            tmp2[:, :, 1:T], x2[:, 16:24, 1:T], x2[:, 24:32, 0 : T - 1]
        )
        nc.vector.tensor_copy(tmp2[:, :, T : T + 1], x2[:, 24:32, T - 1 : T])
        nc.vector.tensor_copy(sig[:, :, 0:2], tmp1[:, :, 0:2])
        nc.vector.tensor_add(
            sig[:, :, 2 : T + 1], tmp1[:, :, 2 : T + 1], tmp2[:, :, 0 : T - 1]
        )
        nc.vector.tensor_copy(
            sig[:, :, T + 1 : T + 3], tmp2[:, :, T - 1 : T + 1]
        )
        nc.sync.dma_start(outre_ap[:], sig[:], max_dma_last_dim=260)


# ---------------------------------------------------------------- build + run
_CACHE = {}
SPAD = 520  # padded s extent of outre (>= T_CORE + 3)


def _build(T):
    import concourse.bacc as bacc
    import concourse.tile as tile
    import concourse.mybir as mybir

    dt = mybir.dt
    SC = (T + 3 + 127) // 128
    nc = bacc.Bacc("TRN2", target_bir_lowering=False, debug=False, num_devices=N_CORES)
    zall_t = nc.dram_tensor("zall", [128, 33, T], dt.bfloat16, kind="ExternalInput")
    zedge_t = nc.dram_tensor("zedge", [2, T], dt.bfloat16, kind="ExternalInput")
    w1_t = nc.dram_tensor("w1", [128, 33, 128], dt.bfloat16, kind="ExternalInput")
    w2p_t = nc.dram_tensor("w2p", [128, 32, 128], dt.bfloat16, kind="ExternalInput")
    wim_t = nc.dram_tensor("wim", [8, 1024], dt.bfloat16, kind="ExternalInput")
    spad = max(SPAD, T + 3)
    outre_t = nc.dram_tensor(
        "outre", [128, 8, spad], dt.bfloat16, kind="ExternalOutput"
    )
    outim_t = nc.dram_tensor(
        "outim", [128, SC, 1024], dt.bfloat16, kind="ExternalOutput"
    )
    with tile.TileContext(nc) as tc:
        emit_kernel(
            tc, outre_t.ap(), outim_t.ap(), zall_t.ap(),
            zedge_t.ap(), w1_t.ap(), w2p_t.ap(), wim_t.ap(), T,
        )
    nc.compile()
    return nc


# outre partition order is p' = 4*q + 2*par + m1h (p = m1 + 64*par,
# m1 = q + 32*m1h); PP[p] = p' un-permutes on the host
PP = np.empty(128, dtype=np.int64)
for _p in range(128):
    _par, _m1 = _p >> 6, _p & 63
    PP[_p] = 4 * (_m1 & 31) + 2 * _par + (_m1 >> 5)


def core_out_to_sig(outre, outim, T):
    """[128,8,spad] bf16 + [128,SC,1024] bf16 -> [2, (T+3)*1024] f32."""
    SB = T + 3
    re = (
        np.asarray(outre, dtype=np.float32)[PP]
        .transpose(2, 1, 0)
        .reshape(-1, 1024)[:SB]
    )
    im = (
        np.asarray(outim, dtype=np.float32)
        .transpose(1, 0, 2)
        .reshape(-1, 1024)[:SB]
    )
    return np.stack([re.reshape(-1), im.reshape(-1)])


_KIDXA = np.arange(33)[None, :] + 64 * np.arange(32)[:, None]  # [32, 33]
_KIDXB = (64 - np.arange(33))[None, :] + 64 * np.arange(32)[:, None]  # [32, 33]


def make_in_maps(z, window):
    """Shard full f32 inputs into per-core bf16 in_maps."""
    zb = np.asarray(z, dtype=np.float32).astype(_bf16)
    wkey = window.tobytes()
    if _CACHE.get("wkey") != wkey:
        _CACHE["weights"] = build_weights(np.asarray(window, dtype=np.float32))
        _CACHE["wkey"] = wkey
    w1, w2p, wim = _CACHE["weights"]
    in_maps = []
    for m in range(N_CORES):
        zc = zb[:, :, m * T_CORE : (m + 1) * T_CORE]
        zall = np.empty((128, 33, T_CORE), dtype=_bf16)
        zall[0:64] = zc[:, _KIDXA, :].reshape(64, 33, T_CORE)
        zall[64:128] = zc[:, _KIDXB, :].reshape(64, 33, T_CORE)
        zedge = np.ascontiguousarray(zc[1, [0, 2048], :])  # [2, T]
        in_maps.append(
            {"zall": zall, "zedge": zedge, "w1": w1, "w2p": w2p, "wim": wim}
        )
    return in_maps


def kernel(z, window):
    from concourse.bass_utils import run_bass_kernel_spmd

    z = np.asarray(z, dtype=np.float32)
    window = np.asarray(window, dtype=np.float32)
    assert z.shape == (2, FREQ, T_FRAMES)

    if "nc" not in _CACHE:
        _CACHE["nc"] = _build(T_CORE)
    nc = _CACHE["nc"]

    in_maps = make_in_maps(z, window)
    res = run_bass_kernel_spmd(nc, in_maps, core_ids=list(range(N_CORES)))

    full = np.zeros((2, L_FULL), dtype=np.float32)
    span = (T_CORE + 3) * 1024
    for m in range(N_CORES):
        o = core_out_to_sig(res.results[m]["outre"], res.results[m]["outim"], T_CORE)
        full[:, m * T_CORE * HOP : m * T_CORE * HOP + span] += o
    out = full[:, N_FFT // 2 : L_FULL - N_FFT // 2]

    win = window.astype(np.float64)
    ws_start = win[0:1024] + win[1024:2048] + win[2048:3072]
    ws_end = win[1024:2048] + win[2048:3072] + win[3072:4096]
    out[:, :1024] *= ((3.0 / 4096.0) / ws_start).astype(np.float32)[None, :]
    out[:, -1024:] *= ((3.0 / 4096.0) / ws_end).astype(np.float32)[None, :]
    return out
